# revision 12
# baseline (speedup 1.0000x reference)
"""Self-contained Trainium2 (Bass/Tile) kernel: single-head causal attention.

Problem: embeddings [4,4096,1024] f32; Wq/Wk/Wv [1024,1024] f32 (torch Linear
layout [out,in]).  out = softmax(causal(QK^T)/sqrt(D)) @ V, computed per batch.

Distribution: 8 NeuronCores, one SPMD program.  Core c handles batch c//2 and
8 query chunks of 256 rows.  Causal load-balance with a uniform program: slot j
processes nkt_j = 4j+4 K-tiles (128 rows each); the even core of a batch pair
takes the chunk at row offset 512j (needs 4j+2 K-tiles; the 2 extra are masked
to zero), the odd core takes offset 512j+256 (needs exactly 4j+4).  Per-core
differences (batch data, q-row selection, causal masks) enter via input data
only, so all 8 cores run the same NEFF.

Algorithm (all model math on device, bf16 matmuls / f32 accumulation):
  M  = Wq^T Wk                  (scores = emb_q M emb_k^T; K never formed)
  qt = M^T embq^T               (kept resident in SBUF)
  s  = emb_k qt                 (per 128-row K-tile)
  w  = exp(s/sqrt(D)) * mask
  l  = ones^T w                 (1-col stationary matmul, accumulated in PSUM)
  WE = Sum_k w_k emb_k          ([d, q] accumulators, 2 d-chunks per PSUM bank)
  out= (WE^T Wv^T) / l          (V projection is folded in AFTER the weighted
                                 sum: (w emb) Wv^T == w (emb Wv^T), halving
                                 the projection work and killing the V stream)
"""

import math
import os
import sys
import types

import numpy as np
import ml_dtypes

B, S, D = 4, 4096, 1024
NCORES = 8
NSLOT = 8
CHUNK = 256          # q rows per slot
NKT = [4 * j + 4 for j in range(NSLOT)]   # K-tiles (128 rows) per slot
INV_SQRT_D = 1.0 / math.sqrt(D)
BF16 = ml_dtypes.bfloat16


# ---------------------------------------------------------------------------
# Environment patches (compiler workarounds + profiling hook shim)
# ---------------------------------------------------------------------------

def _install_patches():
    import json as _json
    import concourse.bass as bass

    if not getattr(bass.Bass, "_mw_patched", False):
        _orig_to_json = bass.Bass.to_json_bytes

        def to_json_bytes(self):
            # This walrus build rejects any instruction carrying more than one
            # sync wait ("Too many sync wait commands").  Split extra waits
            # onto single-wait NoOps inserted just before the instruction (the
            # engine executes them in order, so semantics are unchanged).
            raw = _orig_to_json(self)
            m = _json.loads(raw)
            ctr = 0
            changed = False
            for fn in m.get("functions", []):
                for bb in fn.get("blocks", []):
                    out = []
                    for inst in bb.get("instructions", []):
                        si = inst.get("sync_info")
                        if si:
                            waits = si.get("on_wait") or []
                            if len(waits) > 1:
                                changed = True
                                for w in waits[:-1]:
                                    ctr += 1
                                    out.append({
                                        "debug": inst.get("debug", 0),
                                        "engine": inst["engine"],
                                        "ins": [],
                                        "outs": [],
                                        "name": f"I-mw{ctr}",
                                        "opcode": "NoOp",
                                        "text_hint": "mwsplit",
                                        "sync_info": {"on_wait": [w],
                                                      "on_update": []},
                                    })
                                si["on_wait"] = [waits[-1]]
                        out.append(inst)
                    bb["instructions"] = out
            if not changed:
                return raw
            return _json.dumps(m).encode()

        bass.Bass.to_json_bytes = to_json_bytes
        bass.Bass._mw_patched = True

    # Don't upload NEFF/trace artifacts anywhere; keep them local.
    import concourse.bass_utils as bu
    bu.upload_artifacts = lambda tmpdir: tmpdir


def _install_ntff_hook() -> bool:
    """Register the axon NTFF profiling hook (missing module in this image)."""
    try:
        import antenv.axon_hooks  # noqa: F401
        return True
    except ImportError:
        pass
    try:
        mod = types.ModuleType("antenv.axon_hooks")
        state = {"hook": None}
        mod.set_axon_ntff_profile_hook = lambda h: state.__setitem__("hook", h)
        mod.get_axon_ntff_profile_hook = lambda: state["hook"]
        sys.modules["antenv.axon_hooks"] = mod
        import antenv
        antenv.axon_hooks = mod
        from trn_agent_boot.trn_boot import _ntff_profile_via_ctypes
        mod.set_axon_ntff_profile_hook(
            _ntff_profile_via_ctypes("/opt/axon/libaxon_pjrt.so"))
        return True
    except Exception:
        return False


# ---------------------------------------------------------------------------
# Graph
# ---------------------------------------------------------------------------

def _build_graph():
    import concourse.bass as bass
    import concourse.mybir as mybir
    import concourse.tile as tile

    f32 = mybir.dt.float32
    bf16 = mybir.dt.bfloat16
    Exp = mybir.ActivationFunctionType.Exp
    Copy = mybir.ActivationFunctionType.Copy

    nc = bass.Bass("TRN2", debug=False, num_devices=NCORES)

    embT_in = nc.dram_tensor("embT", [D, S], bf16, kind="ExternalInput")
    embK_in = nc.dram_tensor("embK", [S, D], bf16, kind="ExternalInput")
    embqT_in = nc.dram_tensor("embqT", [D, NSLOT * CHUNK], bf16,
                              kind="ExternalInput")
    wq_in = nc.dram_tensor("wqn", [D, D], bf16, kind="ExternalInput")
    wk_in = nc.dram_tensor("wkn", [D, D], bf16, kind="ExternalInput")
    wvT_in = nc.dram_tensor("wvT", [D, D], bf16, kind="ExternalInput")
    masks_in = nc.dram_tensor("masks", [NSLOT, 4, 128, CHUNK], bf16,
                              kind="ExternalInput")
    out_d = nc.dram_tensor("out", [NSLOT * CHUNK, D], bf16,
                           kind="ExternalOutput")

    with tile.TileContext(nc) as tc:
        with (
            tc.tile_pool(name="wsb", bufs=1) as wsb,          # weights + M
            tc.tile_pool(name="etsb", bufs=1) as etsb,        # emb^T resident
            tc.tile_pool(name="qtsb", bufs=1) as qtsb,        # qt resident
            tc.tile_pool(name="eqs", bufs=2) as eqs,          # embq^T stream
            tc.tile_pool(name="eks", bufs=6) as eks,          # emb K stream
            tc.tile_pool(name="wts", bufs=4) as wts,          # exp weights
            tc.tile_pool(name="mks", bufs=2) as mks,          # mask stream
            tc.tile_pool(name="wes", bufs=2) as wes,          # WE sbuf copies
            tc.tile_pool(name="outs", bufs=2) as outs,        # output stage
            tc.tile_pool(name="smalls", bufs=2) as smalls,    # l / r tiles
            tc.tile_pool(name="pbig", bufs=4, space="PSUM") as pbig,
            tc.tile_pool(name="psc", bufs=3, space="PSUM") as psc,
            tc.tile_pool(name="pl", bufs=1, space="PSUM") as pl_pool,
        ):
            # constants
            ones = smalls.tile([128, 1], bf16, name="ones", tag="ones")
            nc.gpsimd.memset(ones[:], 1.0)

            # resident weight tiles [128, 1024] per 128-row chunk.
            # DMA order = need order: wq/wk gate the M build (first matmuls),
            # embT gates scores, wvT is only needed at the first O-proj.
            wq_n, wk_n, wv_t = [], [], []
            for dc in range(8):
                for lst, src, nm in ((wq_n, wq_in, "wq"), (wk_n, wk_in, "wk")):
                    t = wsb.tile([128, D], bf16, name=f"{nm}{dc}",
                                 tag=f"{nm}{dc}")
                    nc.sync.dma_start(t[:], src[dc * 128:(dc + 1) * 128, :])
                    lst.append(t)

            # resident emb^T tiles [128d, S] (scores lhsT)
            embt_sb = []
            for dc in range(8):
                t = etsb.tile([128, S], bf16, name=f"et{dc}", tag=f"et{dc}")
                nc.sync.dma_start(t[:], embT_in[dc * 128:(dc + 1) * 128, :])
                embt_sb.append(t)
            for dc in range(8):
                t = wsb.tile([128, D], bf16, name=f"wv{dc}", tag=f"wv{dc}")
                nc.sync.dma_start(t[:], wvT_in[dc * 128:(dc + 1) * 128, :])
                wv_t.append(t)

            # ---------------- M = Wq^T @ Wk  [d_a, d_b] ----------------
            # scores = Q K^T = (emb_q Wq^T)(emb_k Wk^T)^T = emb_q M emb_k^T,
            # so the K projection never needs to be materialized.
            m_sb = []
            for ac in range(8):
                t = wsb.tile([128, D], bf16, name=f"m{ac}", tag=f"m{ac}")
                for h in range(2):
                    psum = pbig.tile([128, 512], f32, name=f"pm{ac}_{h}",
                                     tag="big")
                    for ec in range(8):
                        nc.tensor.matmul(
                            psum[:],
                            wq_n[ec][:, ac * 128:(ac + 1) * 128],
                            wk_n[ec][:, h * 512:(h + 1) * 512],
                            start=(ec == 0), stop=(ec == 7))
                    nc.scalar.copy(t[:, h * 512:(h + 1) * 512], psum[:])
                m_sb.append(t)

            # ---------------- qt = M^T @ embq^T  (scores rhs), resident ----
            qt_sb = [[None] * 8 for _ in range(NSLOT)]
            for j in range(NSLOT):
                eblk = []
                for dc in range(8):
                    t = eqs.tile([128, CHUNK], bf16, name=f"eq{j}_{dc}",
                                 tag=f"eq{dc}")
                    nc.sync.dma_start(
                        t[:], embqT_in[dc * 128:(dc + 1) * 128,
                                       j * CHUNK:(j + 1) * CHUNK])
                    eblk.append(t)
                for bc in range(8):
                    psum = pbig.tile([128, 512], f32, name=f"pq{j}_{bc}",
                                     tag="big")
                    for ac in range(8):
                        nc.tensor.matmul(
                            psum[:, 0:CHUNK],
                            m_sb[ac][:, bc * 128:(bc + 1) * 128],
                            eblk[ac][:], start=(ac == 0), stop=(ac == 7))
                    qt = qtsb.tile([128, CHUNK], bf16, name=f"qt{j}_{bc}",
                                   tag=f"qt{j}_{bc}")
                    nc.scalar.copy(qt[:], psum[:, 0:CHUNK])
                    qt_sb[j][bc] = qt

            # ---------------- attention ----------------
            for j in range(NSLOT):
                nkt = NKT[j]
                mask_tiles = []
                for mt in range(4):
                    t = mks.tile([128, CHUNK], bf16, name=f"mk{j}_{mt}",
                                 tag=f"mk{mt}")
                    nc.sync.dma_start(t[:], masks_in[j, mt, :, :])
                    mask_tiles.append(t)

                # l accumulator: [1, 256] on one PSUM bank
                l_ps = pl_pool.tile([1, 512], f32, name=f"l{j}", tag="l")
                # WE accumulators: 8 d-chunks packed 2 per [128,512] bank.
                # The dc-even start=True matmul clears the whole bank
                # (zeroing the dc-odd half), so the dc-odd chain runs
                # start=False from its first matmul.
                we_ps = [pbig.tile([128, 512], f32, name=f"we{j}_{i}",
                                   tag="big") for i in range(4)]

                # Two-kt software pipeline: the wt-consuming matmuls are
                # emitted two score-groups behind their producer, so the
                # in-order PE queue never waits on the exp -> mask-mul chain.
                # Masked k-tiles are processed FIRST so the slot tail (which
                # has no score-group cover left) only waits on a prompt exp.
                LAG = 2
                kt_order = list(range(nkt - 4, nkt)) + list(range(nkt - 4))
                wt_q, ek_q = [], []
                for pos in range(nkt + LAG):
                    if pos < nkt:
                        kt = kt_order[pos]
                        ek = eks.tile([128, D], bf16, name=f"ek{j}_{kt}",
                                      tag="ek")
                        nc.sync.dma_start(ek[:],
                                          embK_in[kt * 128:(kt + 1) * 128, :])
                        ek_q.append(ek)

                        s_ps = psc.tile([128, 512], f32, name=f"s{j}_{kt}",
                                        tag="s")
                        for dc in range(8):
                            nc.tensor.matmul(
                                s_ps[:, 0:CHUNK],
                                embt_sb[dc][:, kt * 128:(kt + 1) * 128],
                                qt_sb[j][dc][:], start=(dc == 0),
                                stop=(dc == 7))

                        wt = wts.tile([128, CHUNK], bf16, name=f"w{j}_{kt}",
                                      tag="wt")
                        nc.scalar.activation(wt[:], s_ps[:, 0:CHUNK], Exp,
                                             bias=0.0, scale=INV_SQRT_D)
                        if kt >= nkt - 4:
                            nc.vector.tensor_mul(wt[:], wt[:],
                                                 mask_tiles[kt - (nkt - 4)][:])
                        wt_q.append(wt)

                    if pos >= LAG:
                        ct = pos - LAG
                        wt_c, ek_c = wt_q[ct], ek_q[ct]
                        first, last = ct == 0, ct == nkt - 1
                        nc.tensor.matmul(l_ps[:, 0:CHUNK], ones[:], wt_c[:],
                                         start=first, stop=last)
                        for dc in range(8):
                            half = dc % 2
                            sl = we_ps[dc // 2][:, half * 256:half * 256 + 256]
                            nc.tensor.matmul(
                                sl, ek_c[:, dc * 128:(dc + 1) * 128], wt_c[:],
                                start=(first and half == 0),
                                stop=(last and half == 1),
                                skip_group_check=True)

                # build 1/l per q partition first (frees the l bank early),
                # then stage WE to SBUF (bf16)
                l_sb = smalls.tile([1, CHUNK], f32, name=f"ls{j}", tag="ls")
                nc.vector.tensor_copy(l_sb[:], l_ps[:, 0:CHUNK])
                r_t = smalls.tile([128, 2], f32, name=f"rt{j}", tag="rt")
                for qs in range(2):
                    nc.sync.dma_start(r_t[:, qs:qs + 1],
                                      l_sb[:, qs * 128:(qs + 1) * 128])
                r_sb = smalls.tile([128, 2], f32, name=f"r{j}", tag="r")
                nc.vector.reciprocal(r_sb[:], r_t[:])
                ws = []
                for dc in range(8):
                    t = wes.tile([128, CHUNK], bf16, name=f"ws{j}_{dc}",
                                 tag=f"ws{dc}")
                    half = dc % 2
                    nc.vector.tensor_copy(
                        t[:], we_ps[dc // 2][:, half * 256:half * 256 + 256])
                    ws.append(t)

                # out = (WE^T @ Wv^T) * (1/l), per 128-q group
                for qs in range(2):
                    o_sb = outs.tile([128, D], bf16, name=f"o{j}_{qs}",
                                     tag="o")
                    for eb in range(2):
                        o_ps = pbig.tile([128, 512], f32,
                                         name=f"po{j}_{qs}_{eb}", tag="big")
                        for dc in range(8):
                            nc.tensor.matmul(
                                o_ps[:],
                                ws[dc][:, qs * 128:(qs + 1) * 128],
                                wv_t[dc][:, eb * 512:(eb + 1) * 512],
                                start=(dc == 0), stop=(dc == 7))
                        nc.scalar.activation(
                            o_sb[:, eb * 512:(eb + 1) * 512], o_ps[:], Copy,
                            bias=0.0, scale=r_sb[:, qs:qs + 1])
                    row = (j * 2 + qs) * 128
                    nc.sync.dma_start(out_d[row:row + 128, :], o_sb[:])

    return nc


_CACHED = {}


def _get_graph():
    if "nc" not in _CACHED:
        _install_patches()
        _CACHED["nc"] = _build_graph()
    return _CACHED["nc"]


# ---------------------------------------------------------------------------
# Host-side staging
# ---------------------------------------------------------------------------

def _offsets(parity):
    return [512 * j + 256 * parity for j in range(NSLOT)]


def _masks(parity):
    m = np.zeros((NSLOT, 4, 128, CHUNK), dtype=np.float32)
    offs = _offsets(parity)
    for j in range(NSLOT):
        for t in range(4):
            kt = NKT[j] - 4 + t
            p = np.arange(128)[:, None]
            x = np.arange(CHUNK)[None, :]
            m[j, t] = ((offs[j] + x) >= (kt * 128 + p)).astype(np.float32)
    return m.astype(BF16)


def kernel(embeddings, Wq, Wk, Wv):
    embeddings = np.asarray(embeddings, dtype=np.float32)
    Wq = np.asarray(Wq, dtype=np.float32)
    Wk = np.asarray(Wk, dtype=np.float32)
    Wv = np.asarray(Wv, dtype=np.float32)

    nc = _get_graph()
    from concourse.bass_utils import run_bass_kernel_spmd

    wqn = Wq.astype(BF16)
    wkn = Wk.astype(BF16)
    wvT = np.ascontiguousarray(Wv.T).astype(BF16)
    masks_by_par = [_masks(0), _masks(1)]

    embT_by_b, embK_by_b = [], []
    for b in range(B):
        emb_b = embeddings[b]
        embT_by_b.append(np.ascontiguousarray(emb_b.T).astype(BF16))
        embK_by_b.append(np.ascontiguousarray(emb_b).astype(BF16))

    in_maps = []
    for c in range(NCORES):
        b, par = divmod(c, 2)
        offs = _offsets(par)
        embT = embT_by_b[b]
        embqT = np.ascontiguousarray(
            np.concatenate([embT[:, g:g + CHUNK] for g in offs], axis=1))
        in_maps.append({
            "embT": embT,
            "embK": embK_by_b[b],
            "embqT": embqT,
            "wqn": wqn,
            "wkn": wkn,
            "wvT": wvT,
            "masks": masks_by_par[par],
        })

    trace = bool(int(os.environ.get("BASS_KERNEL_TRACE", "0")))
    kwargs = {}
    if trace:
        kwargs["trace"] = _install_ntff_hook()

    res = run_bass_kernel_spmd(nc, in_maps, core_ids=list(range(NCORES)),
                               **kwargs)
    _CACHED["last_result"] = res

    out = np.empty((B, S, D), dtype=np.float32)
    for c in range(NCORES):
        b, par = divmod(c, 2)
        offs = _offsets(par)
        core_out = res.results[c]["out"].astype(np.float32)
        for j, g in enumerate(offs):
            out[b, g:g + CHUNK] = core_out[j * CHUNK:(j + 1) * CHUNK]
    return out
